# revision 1
# baseline (speedup 1.0000x reference)
"""Bidirectional 2-layer LSTM (B=256, T=128, EMB=256, HS=512, VS=64) on 8 trn2 cores.

Sharding: 4-way data-parallel over batch x 2-way direction split.
Core c handles batch quarter q=c//2, direction d=c%2 (0=fwd, 1=bwd; bwd cores
get time-reversed input + the W_b* weights, so the NEFF is identical SPMD).

Per-core device program (Tile framework):
  - fused scan over t: layer0 step t and layer1 step t-1 interleaved
    (two independent dependency chains hide per-step latency).
  - gates matmuls: stationary = hT/xT [K=128, M=64-batch] bf16, moving =
    weight tiles [K=128, N=512] bf16, accumulated fp32 in PSUM, 2x column
    tiling (tile_position (0,0)/(0,64)) so both PE array halves run.
  - gate blocks are reordered on host to [i,f,o,g] per hidden-half so each
    PSUM partition half (batch 0:64 / 64:128 <-> hid half 0/1) is a
    self-contained LSTM cell slice: elementwise runs on all 128 partitions.
  - h is transposed each step (PE transpose via identity, or DMA xbar
    transpose) to feed the next step's stationary operand.
  - compress: each core computes its direction's partial compress^T
    PT = WcT_d.T @ h1T in 8-step chunks, AllGathers chunks with its pair
    core, then combines (add + tanh + fc) into logits^T.
"""

import os
import sys
from contextlib import ExitStack

import numpy as np
import ml_dtypes

for _p in ("/opt/trn_rl_repo",):
    if _p not in sys.path and os.path.isdir(_p):
        sys.path.insert(0, _p)

os.environ.setdefault("JAX_COMPILATION_CACHE_DIR", "/tmp/jaxcache")
os.environ.setdefault("JAX_PERSISTENT_CACHE_MIN_COMPILE_TIME_SECS", "1")

B, T, VS, EMB, HS = 256, 128, 64, 256, 512
NCORES = 8
BC = 64          # batch per core
ROWS = T * BC    # 8192 rows of (t, b) per core
G4 = 4 * HS      # 2048 gate dims
CHUNK = 8        # compress chunk: timesteps per PT chunk
NCHUNK = T // CHUNK
XCH = 16         # x-stream chunk (timesteps per input DMA)

BF16 = ml_dtypes.bfloat16

_PAIRS = [[0, 1], [2, 3], [4, 5], [6, 7]]


def _gate_perm():
    """Reorder gate rows from [i,f,g,o] blocks of 512 to per-hid-half
    [i_h, f_h, o_h, g_h] blocks of 256 (half-major)."""
    perm = []
    for h in (0, 1):
        for blk in (0, 1, 3, 2):  # i, f, o, g in original block order
            base = 512 * blk + 256 * h
            perm.extend(range(base, base + 256))
    return np.array(perm)


def build_program(with_gate_bias0, with_gate_bias1, transpose_mode="pe", t_steps=T, repeat=1):
    import concourse.bass as bass  # noqa: F401
    import concourse.mybir as mybir
    import concourse.tile as tile
    from concourse import bacc

    f32 = mybir.dt.float32
    bf16 = mybir.dt.bfloat16
    AF = mybir.ActivationFunctionType
    Tn = t_steps
    rows = Tn * BC
    nchunk = Tn // CHUNK

    nc = bacc.Bacc()

    # ---- I/O ----
    ohT = nc.dram_tensor("ohT", [64, rows], bf16, kind="ExternalInput")
    g0tab = nc.dram_tensor("g0tab", [64, G4], bf16, kind="ExternalInput")
    wh0 = nc.dram_tensor("wh0", [4, 128, G4], bf16, kind="ExternalInput")
    wx1 = nc.dram_tensor("wx1", [4, 128, G4], bf16, kind="ExternalInput")
    wh1 = nc.dram_tensor("wh1", [4, 128, G4], bf16, kind="ExternalInput")
    wc = nc.dram_tensor("wc", [4, 128, 512], bf16, kind="ExternalInput")
    fct = nc.dram_tensor("fct", [4, 128, 64], bf16, kind="ExternalInput")
    cbias = nc.dram_tensor("cbias", [4, 128, 1], f32, kind="ExternalInput")
    fbias = nc.dram_tensor("fbias", [64, 1], f32, kind="ExternalInput")
    ident = nc.dram_tensor("ident", [128, 128], bf16, kind="ExternalInput")
    if with_gate_bias1:
        gb1 = nc.dram_tensor("gb1", [1, G4], bf16, kind="ExternalInput")
    logT = nc.dram_tensor("logT", [64, rows], f32, kind="ExternalOutput")

    # internal DRAM for the pair exchange
    pt_self = nc.dram_tensor("pt_self", [nchunk, 4, 128, 512], bf16)
    pt_both = nc.dram_tensor("pt_both", [nchunk, 2, 4, 128, 512], bf16)

    if os.environ.get("BLSTM_NULL", "0") == "1":
        with tile.TileContext(nc) as tc, ExitStack() as ctx:
            pool = ctx.enter_context(tc.tile_pool(name="np", bufs=1))
            z = pool.tile([64, 512], f32, name="z")
            nc.vector.memset(z, 0.0)
            nc.sync.dma_start(out=logT[:, 0:512], in_=z)
        nc.finalize()
        return nc

    with tile.TileContext(nc) as tc, ExitStack() as ctx:
        wpool = ctx.enter_context(tc.tile_pool(name="weights", bufs=1))
        spool = ctx.enter_context(tc.tile_pool(name="state", bufs=1))
        xpool = ctx.enter_context(tc.tile_pool(name="xin", bufs=2))
        work = ctx.enter_context(tc.tile_pool(name="work", bufs=2))
        g0pool = ctx.enter_context(tc.tile_pool(name="g0p", bufs=1, space="PSUM"))
        g1pool = ctx.enter_context(tc.tile_pool(name="g1p", bufs=1, space="PSUM"))
        trpool = ctx.enter_context(tc.tile_pool(name="trp", bufs=2, space="PSUM"))
        auxp = ctx.enter_context(tc.tile_pool(name="auxp", bufs=2, space="PSUM"))

        # ---- load weights ----
        def load(dram, n, cols, dt=bf16, tag=None):
            tiles = []
            for k in range(n):
                t_ = wpool.tile([128, cols], dt, tag=f"{tag}{k}", name=f"{tag}{k}")
                nc.sync.dma_start(out=t_, in_=dram[k])
                tiles.append(t_)
            return tiles

        g0tab_s = wpool.tile([64, G4], bf16, tag="g0tab")
        nc.sync.dma_start(out=g0tab_s, in_=g0tab[:, :])
        wh0_s = load(wh0, 4, G4, tag="wh0")
        wx1_s = load(wx1, 4, G4, tag="wx1")
        wh1_s = load(wh1, 4, G4, tag="wh1")
        wc_s = load(wc, 4, 512, tag="wc")
        fct_s = load(fct, 4, 64, tag="fct")
        cbias_s = wpool.tile([128, 4], f32, tag="cbias")
        for oc in range(4):
            nc.sync.dma_start(out=cbias_s[:, oc : oc + 1], in_=cbias[oc])
        fbias_s = wpool.tile([64, 1], f32, tag="fbias")
        nc.sync.dma_start(out=fbias_s, in_=fbias[:, :])
        ident_s = wpool.tile([128, 128], bf16, tag="ident")
        nc.sync.dma_start(out=ident_s, in_=ident[:, :])
        if with_gate_bias1:
            gb1_s = wpool.tile([1, G4], bf16, tag="gb1")
            nc.sync.dma_start(out=gb1_s, in_=gb1[:, :])
        ones_s = None
        if with_gate_bias1:
            ones_s = wpool.tile([1, 64], bf16, tag="ones")
            nc.vector.memset(ones_s, 1.0)

        # ---- state ----
        h0T_ring = [spool.tile([128, 256], bf16, tag=f"h0T{i}", name=f"h0T{i}") for i in range(3)]
        h1tc = [spool.tile([128, CHUNK * 256], bf16, tag=f"h1tc{i}", name=f"h1tc{i}") for i in range(2)]
        h1T_init = spool.tile([128, 256], bf16, tag="h1Tinit")
        cst = [
            [spool.tile([128, 256], f32, tag=f"c{l}{i}", name=f"c{l}{i}") for i in range(2)]
            for l in (0, 1)
        ]
        def init_state():
            for t_ in h0T_ring:
                nc.vector.memset(t_, 0.0)
            nc.vector.memset(h1T_init, 0.0)
            for l in (0, 1):
                nc.vector.memset(cst[l][0], 0.0)

        CHUNKCOL = {0: 0, 2: 64, 1: 128, 3: 192}

        xa_tiles = {}
        h0_tiles = {}
        h1_tiles = {}

        def gates_matmuls(gp, x_chunks, h_prev, wx_t, wh_t, gb_t):
            """Column-tiled, K-accumulated gate matmuls. Emission is
            k-outer with the two col-tiles adjacent so they run
            concurrently on the PE array (different col groups)."""
            stats = [(xt_[:, off : off + 64], wx_t[i]) for i, (xt_, off) in enumerate(x_chunks)]
            stats += [
                (h_prev[:, CHUNKCOL[kc] : CHUNKCOL[kc] + 64], wh_t[kc]) for kc in range(4)
            ]
            if gb_t is not None:
                stats.append((ones_s, gb_t))
            nk = len(stats)
            # Two phases; within a phase the two regions live in different
            # PSUM banks AND different PE col-groups, so the interleaved
            # matmuls run concurrently and the start=True bank-clears of
            # one region cannot wipe a live accumulation in the other.
            for phase in (((0, 0), (1, 1)), ((0, 1), (1, 0))):
                for kid, (lhs, w) in enumerate(stats):
                    for ct, n in phase:
                        nc.tensor.matmul(
                            gp[64 * ct : 64 * ct + 64, 512 * n : 512 * n + 512],
                            lhsT=lhs,
                            rhs=w[:, ct * 1024 + n * 512 : ct * 1024 + n * 512 + 512],
                            start=(kid == 0),
                            stop=(kid == nk - 1),
                            tile_position=(0, 64 * ct),
                        )

        def cell(layer, gp, t):
            S = work.tile([128, 768], bf16, tag=f"S{layer}")
            nc.scalar.activation(S, gp[:, 0:768], AF.Sigmoid)
            G2 = work.tile([128, 256], bf16, tag=f"G2{layer}")
            nc.scalar.activation(G2, gp[:, 768:1024], AF.Tanh)
            c_prev = cst[layer][t % 2]
            c_new = cst[layer][(t + 1) % 2]
            prod = work.tile([128, 512], f32, tag=f"prod{layer}")
            nc.vector.tensor_mul(prod[:, 0:256], S[:, 0:256], G2)
            nc.vector.tensor_mul(prod[:, 256:512], S[:, 256:512], c_prev)
            nc.vector.tensor_add(c_new, prod[:, 0:256], prod[:, 256:512])
            TC = work.tile([128, 256], bf16, tag=f"TC{layer}")
            nc.scalar.activation(TC, c_new, AF.Tanh)
            H = work.tile([128, 256], bf16, tag=f"H{layer}")
            nc.vector.tensor_mul(H, S[:, 512:768], TC)
            return H

        def transpose_h(H, dest, layer):
            use_dma = transpose_mode == "dma" or (transpose_mode == "hybrid" and layer == 1)
            if use_dma:
                for c in (0, 1):
                    nc.sync.dma_start_transpose(
                        out=dest[:, 128 * c : 128 * c + 128],
                        in_=H[:, 128 * c : 128 * c + 128],
                    )
            else:
                tp_ps = trpool.tile([128, 256], bf16, tag="trps")
                for c in (0, 1):
                    nc.tensor.transpose(
                        out=tp_ps[:, 128 * c : 128 * c + 128],
                        in_=H[:, 128 * c : 128 * c + 128],
                        identity=ident_s,
                    )
                nc.vector.tensor_copy(dest, tp_ps)

        def load_x_chunk(ci):
            if ci * XCH >= Tn or ci in xa_tiles:
                return
            xa = xpool.tile([64, XCH * 64], bf16, tag="xa", name="xa")
            nc.sync.dma_start(
                out=xa, in_=ohT[:, ci * XCH * 64 : (ci * XCH + XCH) * 64]
            )
            xa_tiles[ci] = xa

        def l0_mms(t):
            s = t % XCH
            xa = xa_tiles[t // XCH]
            gp = g0pool.tile([128, 1024], f32, tag="g0", name="g0")
            h_prev = h0T_ring[(t - 1) % 3] if t > 0 else h0T_ring[2]
            x_chunks = [(xa, s * 64)]
            gates_matmuls(gp, x_chunks, h_prev, [g0tab_s], wh0_s, None)
            return gp

        def l1_mms(t):
            gp = g1pool.tile([128, 1024], f32, tag="g1", name="g1")
            h0 = h0T_ring[t % 3]
            if t > 0:
                u = t - 1
                h1_prev = h1tc[(u // CHUNK) % 2][:, (u % CHUNK) * 256 : (u % CHUNK) * 256 + 256]
            else:
                h1_prev = h1T_init
            x_chunks = [(h0, CHUNKCOL[kc]) for kc in range(4)]
            gates_matmuls(gp, x_chunks, h1_prev, wx1_s, wh1_s,
                          gb1_s if with_gate_bias1 else None)
            return gp

        def compress_chunk(c):
            src = h1tc[c % 2].rearrange("p (s k b) -> p s k b", s=CHUNK, k=4, b=64)
            SLOT = {0: 0, 1: 2, 2: 1, 3: 3}
            for oa, ob in ((0, 1), (2, 3)):
                pA = auxp.tile([128, 512], f32, tag="aux", name="pA")
                pB = auxp.tile([128, 512], f32, tag="aux", name="pB")
                # interleave the two oc's with opposite col-tiles: different
                # PSUM banks and different PE col-groups -> concurrent.
                for phase in (((oa, pA, 0), (ob, pB, 1)), ((oa, pA, 1), (ob, pB, 0))):
                    for kc in range(4):
                        for oc, pt, ct in phase:
                            nc.tensor.matmul(
                                pt[64 * ct : 64 * ct + 64, :],
                                lhsT=wc_s[kc][:, oc * 128 + 64 * ct : oc * 128 + 64 * ct + 64],
                                rhs=src[:, :, SLOT[kc], :],
                                start=(kc == 0),
                                stop=(kc == 3),
                                tile_position=(0, 64 * ct),
                            )
                for oc, pt in ((oa, pA), (ob, pB)):
                    pts = work.tile([128, 512], bf16, tag="pts", name="pts")
                    nc.vector.tensor_copy(pts, pt)
                    nc.sync.dma_start(out=pt_self[c, oc], in_=pts)
            if os.environ.get("BLSTM_NO_CC", "0") == "1":
                for oc in range(4):
                    nc.sync.dma_start(out=pt_both[c, 0, oc], in_=pt_self[c, oc])
                    nc.sync.dma_start(out=pt_both[c, 1, oc], in_=pt_self[c, oc])
            else:
                nc.gpsimd.collective_compute(
                    "AllGather",
                    mybir.AluOpType.bypass,
                    replica_groups=_PAIRS,
                    ins=[pt_self[c]],
                    outs=[pt_both[c]],
                )

        def combine_chunk(j):
            comp = []
            for oc in range(4):
                af = work.tile([128, 512], bf16, tag="af")
                nc.sync.dma_start(out=af, in_=pt_both[j, 0, oc])
                ab = work.tile([128, 512], bf16, tag="ab")
                for tl in range(CHUNK):
                    nc.sync.dma_start(
                        out=ab[:, 64 * tl : 64 * tl + 64],
                        in_=pt_both[nchunk - 1 - j, 1, oc, :, 64 * (CHUNK - 1 - tl) : 64 * (CHUNK - tl)],
                    )
                sm = work.tile([128, 512], bf16, tag="sm")
                nc.vector.tensor_add(sm, af, ab)
                cT = work.tile([128, 512], bf16, tag=f"cT{oc}")
                nc.scalar.activation(cT, sm, AF.Tanh, bias=cbias_s[:, oc : oc + 1])
                comp.append(cT)
            lgp = auxp.tile([64, 512], f32, tag="aux", name="lgp")
            for kc in range(4):
                nc.tensor.matmul(
                    lgp,
                    lhsT=fct_s[kc],
                    rhs=comp[kc],
                    start=(kc == 0),
                    stop=(kc == 3),
                    tile_position=(0, 0),
                )
            lgs = work.tile([64, 512], f32, tag="lgs")
            nc.scalar.activation(lgs, lgp, AF.Identity, bias=fbias_s[:, 0:1])
            nc.sync.dma_start(out=logT[:, 512 * j : 512 * (j + 1)], in_=lgs)

        # ---- main fused loop ----
        # Iteration t emits: L0 matmuls(t) | h1-transpose(t-2) | L1 matmuls(t-1)
        # | L0 cell(t) | L1 cell(t-1) | h0-transpose(t) | compress/AG/combines.
        # Transposes are placed so the PE never waits on a cell chain that
        # has not had time to drain; combines trail their AllGathers by two
        # chunks so the PE does not stall on collective latency.
        def ready_at(j):
            return max(j, nchunk - 1 - j)

        def emit_pass():
            combined = set()
            xa_tiles.clear()
            h0_tiles.clear()
            h1_tiles.clear()
            init_state()
            load_x_chunk(0)
            for t in range(Tn + 3):
                if t < Tn:
                    if t % XCH == XCH // 2:
                        load_x_chunk(t // XCH + 1)
                    gp0 = l0_mms(t)
                if 2 <= t < Tn + 2:
                    u = t - 2
                    dst = h1tc[(u // CHUNK) % 2][:, (u % CHUNK) * 256 : (u % CHUNK) * 256 + 256]
                    transpose_h(h1_tiles.pop(u), dst, 1)
                if 1 <= t < Tn + 1:
                    gp1 = l1_mms(t - 1)
                if t < Tn:
                    h0_tiles[t] = cell(0, gp0, t)
                if 1 <= t < Tn + 1:
                    h1_tiles[t - 1] = cell(1, gp1, t - 1)
                if t < Tn:
                    transpose_h(h0_tiles.pop(t), h0T_ring[t % 3], 0)
                if t >= 9 and (t - 9) % CHUNK == 0:
                    c = (t - 9) // CHUNK
                    compress_chunk(c)
                    for j in range(nchunk):
                        if j not in combined and ready_at(j) == c - 2:
                            combined.add(j)
                            combine_chunk(j)
            for j in sorted(set(range(nchunk)) - combined, key=ready_at):
                combine_chunk(j)

        for _ in range(repeat):
            emit_pass()

    nc.finalize()
    return nc


_prog_cache = {}


def _get_program(key):
    if key not in _prog_cache:
        _prog_cache[key] = build_program(*key)
    return _prog_cache[key]


def _prep_core_inputs(x, emb_table, Ws, bs, compress_W, compress_b, fc_W, fc_b,
                      quarter, direction, t_steps=T):
    """Build the per-core input map (numpy)."""
    perm = _gate_perm()
    xq = np.asarray(x[quarter * BC : (quarter + 1) * BC, :t_steps]).astype(np.int64)
    if direction == 1:
        xq = xq[:, ::-1]
    # one-hot^T: ohT[v, t*64+b] = (x[b,t_scan] == v)
    xs = xq.T.reshape(-1)                     # [Tn*BC] token ids, (t,b) order
    ohv = np.zeros((64, t_steps * BC), dtype=np.float32)
    ohv[xs, np.arange(t_steps * BC)] = 1.0

    W0, W1 = Ws
    b0, b1 = bs
    W0r = np.asarray(W0)[perm]                # [2048, EMB+HS]
    W1r = np.asarray(W1)[perm]                # [2048, 2*HS]
    # vocab gate table: G0[v] = emb_table[v] @ W0x^T + b0  (layer-0 x-part + bias)
    g0v = np.asarray(emb_table, dtype=np.float32) @ W0r[:, :EMB].T.astype(np.float32)
    g0v = g0v + np.asarray(b0, dtype=np.float32)[perm][None, :]
    wh0v = W0r[:, EMB:].T.reshape(4, 128, G4)
    wx1v = W1r[:, :HS].T.reshape(4, 128, G4)
    wh1v = W1r[:, HS:].T.reshape(4, 128, G4)

    Wc_d = np.asarray(compress_W)[:, direction * HS : (direction + 1) * HS]
    wcv = Wc_d.T.reshape(4, 128, 512)         # [in-hid, out]
    fctv = np.asarray(fc_W).T.reshape(4, 128, 64)
    cbv = np.asarray(compress_b, dtype=np.float32).reshape(4, 128, 1)
    fbv = np.asarray(fc_b, dtype=np.float32).reshape(64, 1)

    identv = np.eye(128, dtype=np.float32)

    inmap = {
        "ohT": ohv.astype(BF16),
        "g0tab": g0v.astype(BF16),
        "wh0": wh0v.astype(BF16),
        "wx1": wx1v.astype(BF16),
        "wh1": wh1v.astype(BF16),
        "wc": wcv.astype(BF16),
        "fct": fctv.astype(BF16),
        "cbias": cbv,
        "fbias": fbv,
        "ident": identv.astype(BF16),
    }
    if np.any(np.asarray(b1)):
        inmap["gb1"] = np.asarray(b1)[perm].reshape(1, G4).astype(BF16)
    return inmap


def _run(inputs, trace=False, t_steps=T):
    from concourse.bass_utils import run_bass_kernel_spmd

    x = np.asarray(inputs["x"])
    emb_table = np.asarray(inputs["emb_table"], dtype=np.float32)
    with_gb0 = False
    with_gb1 = bool(np.any(np.asarray(inputs["b_f1"])) or np.any(np.asarray(inputs["b_b1"])))
    tmode = os.environ.get("BLSTM_TRANSPOSE", "hybrid")
    rep = int(os.environ.get("BLSTM_REPEAT", "1"))
    nc = _get_program((with_gb0, with_gb1, tmode, t_steps, rep))

    in_maps = []
    for core in range(NCORES):
        q, d = core // 2, core % 2
        Ws = (
            (inputs["W_f0"], inputs["W_f1"]) if d == 0 else (inputs["W_b0"], inputs["W_b1"])
        )
        bs = (
            (inputs["b_f0"], inputs["b_f1"]) if d == 0 else (inputs["b_b0"], inputs["b_b1"])
        )
        im = _prep_core_inputs(
            x, emb_table, Ws, bs, inputs["compress_W"], inputs["compress_b"],
            inputs["fc_W"], inputs["fc_b"], q, d, t_steps,
        )
        if with_gb1 and "gb1" not in im:
            im["gb1"] = np.zeros((1, G4), dtype=BF16)
        in_maps.append(im)

    res = run_bass_kernel_spmd(nc, in_maps, core_ids=list(range(NCORES)), trace=trace)

    out = np.empty((B, t_steps, VS), dtype=np.float32)
    for q in range(4):
        logT = res.results[2 * q]["logT"]    # [64, rows] from the fwd core of pair q
        out[q * BC : (q + 1) * BC] = (
            logT.reshape(VS, t_steps, BC).transpose(2, 1, 0)
        )
    return out, res


def kernel(**inputs):
    out, _ = _run(inputs, trace=False)
    return out


def kernel_profiled(**inputs):
    out, res = _run(inputs, trace=True)
    return out, res



# revision 9
# speedup vs baseline: 5.6553x; 5.6553x over previous
"""Bidirectional 2-layer LSTM (B=256, T=128, EMB=256, HS=512, VS=64) on 8 trn2 cores.

Sharding: core = (batch-half bh, direction d, gate-half g).
Each core runs the full 2-layer recurrence for its 128 batch rows and
direction, computing HALF the gate/hidden dims (hid [g*256,(g+1)*256) of
both layers); gate-half pairs AllGather their h-halves every step.

Per-iteration u (uniform, u = 0..T):
  - 26 matmuls (K<=128, M=128, N=512) accumulate L0 gates (step u) and
    L1 gates (step u-1) into one PSUM tile [128, 2048] =
    [i0 f0 o0 g0 | i1 f1 o1 g1] (own hid-half, 256 each).
  - 6 fused elementwise ops compute BOTH layers' LSTM cells via strided
    3-D APs (sigmoid, tanh(g), i*g|f*c, add, tanh(c), o*tc).
  - H [128, 512] = [H0own|H1own] -> dram -> pair AllGather -> load full
    H [128,1024] -> ONE dma_start_transpose -> hT blocks for next step.
Tail: dir-pairs AllReduce(add) the late h-slot window; partner stream =
sum - own (keeps SPMD code uniform under time reversal); each core
computes compress+fc for its own-scan steps [0, T/2) (fwd cores cover
original t in [0,T/2), bwd cores cover [T/2, T)).
"""

import os
import sys
from contextlib import ExitStack

import numpy as np
import ml_dtypes

for _p in ("/opt/trn_rl_repo",):
    if _p not in sys.path and os.path.isdir(_p):
        sys.path.insert(0, _p)

os.environ.setdefault("JAX_COMPILATION_CACHE_DIR", "/tmp/jaxcache")
os.environ.setdefault("JAX_PERSISTENT_CACHE_MIN_COMPILE_TIME_SECS", "1")

B, T, VS, EMB, HS = 256, 128, 64, 256, 512
NCORES = 8
BC = 128           # batch rows per core
HH = 256           # hid per gate-half

BF16 = ml_dtypes.bfloat16

# core = bh*4 + d*2 + g
GATE_PAIRS = [[0, 1], [2, 3], [4, 5], [6, 7]]   # (bh,d,g0) <-> (bh,d,g1)
DIR_PAIRS = [[0, 2], [1, 3], [4, 6], [5, 7]]    # (bh,f,g) <-> (bh,b,g)

# dmat block j of transposed full-H [128, 8, 128]:
# HF = [H0(g0) H1(g0) H0(g1) H1(g1)] in 256-col slabs ->
# blocks: [h0c0 h0c1 h1c0 h1c1 h0c2 h0c3 h1c2 h1c3]
BH0 = [0, 1, 4, 5]
BH1 = [2, 3, 6, 7]


def build_program(t_steps=T, repeat=1, with_b1=False):
    import concourse.bass as bass  # noqa: F401
    import concourse.mybir as mybir
    import concourse.tile as tile
    from concourse import bacc

    f32 = mybir.dt.float32
    bf16 = mybir.dt.bfloat16
    AF = mybir.ActivationFunctionType
    Tn = t_steps
    TS = Tn // 2                  # own-scan tail steps per core
    NG = TS // 4                  # tail groups (4 steps = 512 rows each)

    nc = bacc.Bacc()

    # ---- I/O ----
    ohT = nc.dram_tensor("ohT", [64, (Tn + 1) * BC], bf16, kind="ExternalInput")
    g0tab = nc.dram_tensor("g0tab", [64, 1024], bf16, kind="ExternalInput")
    wh0 = nc.dram_tensor("wh0", [4, 128, 1024], bf16, kind="ExternalInput")
    wx1 = nc.dram_tensor("wx1", [4, 128, 1024], bf16, kind="ExternalInput")
    wh1 = nc.dram_tensor("wh1", [4, 128, 1024], bf16, kind="ExternalInput")
    wco = nc.dram_tensor("wco", [4, 128, 512], bf16, kind="ExternalInput")
    wcp = nc.dram_tensor("wcp", [4, 128, 512], bf16, kind="ExternalInput")
    fct = nc.dram_tensor("fct", [128, 256], bf16, kind="ExternalInput")
    cbias = nc.dram_tensor("cbias", [128, 4], f32, kind="ExternalInput")
    fbias = nc.dram_tensor("fbias", [64, 1], f32, kind="ExternalInput")
    if with_b1:
        b1row = nc.dram_tensor("b1row", [1, 1024], bf16, kind="ExternalInput")
    logT = nc.dram_tensor("logT", [64, TS * BC], f32, kind="ExternalOutput")

    # internal dram
    hin = nc.dram_tensor("hin", [Tn + 1, 128, 512], bf16)
    hout = nc.dram_tensor("hout", [Tn + 1, 2, 128, 512], bf16)
    arout = nc.dram_tensor("arout", [TS, 2, 128, 512], bf16)

    if os.environ.get("BLSTM_NULL", "0") == "1":
        with tile.TileContext(nc) as tc, ExitStack() as ctx:
            pool = ctx.enter_context(tc.tile_pool(name="np", bufs=1))
            z = pool.tile([64, 512], f32, name="z")
            nc.vector.memset(z, 0.0)
            nc.sync.dma_start(out=logT[:, 0:512], in_=z)
        nc.finalize()
        return nc

    with tile.TileContext(nc) as tc, ExitStack() as ctx:
        wpool = ctx.enter_context(tc.tile_pool(name="weights", bufs=1))
        spool = ctx.enter_context(tc.tile_pool(name="state", bufs=1))
        work = ctx.enter_context(tc.tile_pool(name="work", bufs=1))
        gpool = ctx.enter_context(tc.tile_pool(name="gp", bufs=1, space="PSUM"))
        auxp = ctx.enter_context(tc.tile_pool(name="auxp", bufs=1, space="PSUM"))

        # ---- load weights ----
        def load(dram, n, cols, tag):
            tiles = []
            for k in range(n):
                t_ = wpool.tile([128, cols], bf16, tag=f"{tag}{k}", name=f"{tag}{k}")
                nc.sync.dma_start(out=t_, in_=dram[k])
                tiles.append(t_)
            return tiles

        ohT_s = wpool.tile([64, (Tn + 1) * BC], bf16, tag="ohT")
        nc.sync.dma_start(out=ohT_s, in_=ohT[:, :])
        g0tab_s = wpool.tile([64, 1024], bf16, tag="g0tab")
        nc.sync.dma_start(out=g0tab_s, in_=g0tab[:, :])
        wh0_s = load(wh0, 4, 1024, "wh0")
        wx1_s = load(wx1, 4, 1024, "wx1")
        wh1_s = load(wh1, 4, 1024, "wh1")
        wco_s = load(wco, 4, 512, "wco")
        wcp_s = load(wcp, 4, 512, "wcp")
        fct_s = wpool.tile([128, 256], bf16, tag="fct")
        nc.sync.dma_start(out=fct_s, in_=fct[:, :])
        cbias_s = wpool.tile([128, 4], f32, tag="cbias")
        nc.sync.dma_start(out=cbias_s, in_=cbias[:, :])
        fbias_s = wpool.tile([64, 1], f32, tag="fbias")
        nc.sync.dma_start(out=fbias_s, in_=fbias[:, :])
        if with_b1:
            b1_s = wpool.tile([1, 1024], bf16, tag="b1row")
            nc.sync.dma_start(out=b1_s, in_=b1row[:, :])
            ones_s = wpool.tile([1, 128], bf16, tag="ones")
            nc.vector.memset(ones_s, 1.0)

        # ---- state ----
        # X: [G0(256) c0(256) G1(256) c1(256)] fp32
        X = spool.tile([128, 1024], f32, tag="X")
        hT_ring = [spool.tile([128, 1024], bf16, tag=f"hT{i}", name=f"hT{i}")
                   for i in range(3)]

        def init_state():
            nc.vector.memset(X, 0.0)
            nc.vector.memset(hT_ring[2], 0.0)

        def gates_mms(gp, u, hT):
            """26 (28 with b1) matmuls: L0 gates (step u) into cols 0:1024,
            L1 gates (step u-1) into cols 1024:2048."""
            # (lhsT, [(col_lo, n_tiles_rhs, rhs_tile), ...])
            # regions: r0 = L0 (rhs cols of wh0/g0tab), r1 = L1
            # Build flat list per region so start/stop flags are right.
            reg0 = [(ohT_s[:, u * BC:(u + 1) * BC], g0tab_s)]
            reg0 += [(hT[:, BH0[kc] * 128:BH0[kc] * 128 + 128], wh0_s[kc])
                     for kc in range(4)]
            reg1 = [(hT[:, BH0[kc] * 128:BH0[kc] * 128 + 128], wx1_s[kc])
                    for kc in range(4)]
            reg1 += [(hT[:, BH1[kc] * 128:BH1[kc] * 128 + 128], wh1_s[kc])
                     for kc in range(4)]
            if with_b1:
                reg1.append((ones_s, b1_s))
            for base, stats in ((0, reg0), (1024, reg1)):
                nk = len(stats)
                for kid, (lhs, w) in enumerate(stats):
                    for n in (0, 1):
                        nc.tensor.matmul(
                            gp[:, base + 512 * n: base + 512 * n + 512],
                            lhsT=lhs,
                            rhs=w[:, 512 * n: 512 * n + 512],
                            start=(kid == 0),
                            stop=(kid == nk - 1),
                            tile_position=(0, 0),
                        )

        def cell(gp):
            """Fused 2-layer cell; returns H [128, 512] = [H0own|H1own]."""
            gv = gp.rearrange("p (j c) -> p j c", j=2)       # [128, 2, 1024]
            S = work.tile([128, 1536], bf16, tag="S")
            Sv = S.rearrange("p (j c) -> p j c", j=2)        # [128, 2, 768]
            nc.scalar.activation(Sv, gv[:, :, 0:768], AF.Sigmoid)
            Xv = X.rearrange("p (j c) -> p j c", j=2)        # [128, 2, 512]
            nc.scalar.activation(Xv[:, :, 0:256], gv[:, :, 768:1024], AF.Tanh)
            P = work.tile([128, 1024], f32, tag="P")
            Pv = P.rearrange("p (j c) -> p j c", j=2)        # [128, 2, 512]
            nc.vector.tensor_mul(Pv, Sv[:, :, 0:512], Xv)
            nc.vector.tensor_add(Xv[:, :, 256:512], Pv[:, :, 0:256],
                                 Pv[:, :, 256:512])
            TC = work.tile([128, 512], bf16, tag="TC")
            TCv = TC.rearrange("p (j c) -> p j c", j=2)      # [128, 2, 256]
            nc.scalar.activation(TCv, Xv[:, :, 256:512], AF.Tanh)
            H = work.tile([128, 512], bf16, tag="H")
            Hv = H.rearrange("p (j c) -> p j c", j=2)
            nc.vector.tensor_mul(Hv, Sv[:, :, 512:768], TCv)
            return H

        def emit_recurrence():
            init_state()
            for u in range(Tn + 1):
                gp = gpool.tile([128, 2048], f32, tag="gp", name="gp")
                gates_mms(gp, u, hT_ring[(u - 1) % 3] if u > 0 else hT_ring[2])
                H = cell(gp)
                nc.sync.dma_start(out=hin[u], in_=H)
                nc.gpsimd.collective_compute(
                    "AllGather", mybir.AluOpType.bypass,
                    replica_groups=GATE_PAIRS,
                    ins=[hin[u]], outs=[hout[u]])
                HF = work.tile([128, 1024], bf16, tag="HF")
                for j in (0, 1):
                    nc.sync.dma_start(out=HF[:, j * 512:(j + 1) * 512],
                                      in_=hout[u, j])
                nc.sync.dma_start_transpose(
                    out=hT_ring[u % 3].rearrange("p (j b) -> p j b", j=8),
                    in_=HF)

        def emit_tail():
            # AllReduce the late window: slots [TS+1, Tn+1) (TS slots).
            nc.gpsimd.collective_compute(
                "AllReduce", mybir.AluOpType.add,
                replica_groups=DIR_PAIRS,
                ins=[hin_window()], outs=[arout[:, :, :, :]])
            for i in range(NG):
                emit_tail_group(i)

        def hin_window():
            # copy hout slots [TS+1 : Tn+1) to arin via dram->dram dma?
            # cheaper: AllReduce directly on a hout slice.
            return hout[TS + 1: Tn + 1]

        def emit_tail_group(i):
            # own-scan steps s = 4i .. 4i+3; 512 rows.
            # Load FULL h slots [2,128,512] -> [128, 1024] per step
            # (h1 blocks selected post-transpose via BH1 strides).
            HT1 = work.tile([128, 4096], bf16, tag="HT1")   # own-dir rows
            ART = work.tile([128, 4096], bf16, tag="ART")   # fwd+bwd sum
            OWT = work.tile([128, 4096], bf16, tag="OWT")   # own window
            for k in range(4):
                s = 4 * i + k
                for j in (0, 1):
                    cs = slice((k * 2 + j) * 512, (k * 2 + j + 1) * 512)
                    nc.sync.dma_start(out=HT1[:, cs], in_=hout[s + 1, j])
                    nc.sync.dma_start(out=ART[:, cs], in_=arout[TS - 1 - s, j])
                    nc.sync.dma_start(out=OWT[:, cs], in_=hout[Tn - s, j])
            PRT = work.tile([128, 4096], bf16, tag="PRT")   # partner rows
            nc.vector.tensor_sub(PRT, ART, OWT)
            hTo = work.tile([128, 4096], bf16, tag="hTo")
            nc.sync.dma_start_transpose(
                out=hTo.rearrange("p (j b) -> p j b", j=32), in_=HT1)
            hTp = work.tile([128, 4096], bf16, tag="hTp")
            nc.sync.dma_start_transpose(
                out=hTp.rearrange("p (j b) -> p j b", j=32), in_=PRT)
            # compress matmuls: PT[cc] [128, 512] for 4 comp chunks
            PT = auxp.tile([128, 2048], f32, tag="aux", name="PT")
            hToV = hTo.rearrange("p (s j b) -> p s j b", s=4, j=8)
            hTpV = hTp.rearrange("p (s j b) -> p s j b", s=4, j=8)
            for cc in range(4):
                for ki, (wt, hv) in enumerate(
                        [(wco_s, hToV), (wcp_s, hTpV)]):
                    for kc in range(4):
                        nc.tensor.matmul(
                            PT[:, cc * 512:(cc + 1) * 512],
                            lhsT=wt[kc][:, cc * 128:(cc + 1) * 128],
                            rhs=hv[:, :, BH1[kc], :],
                            start=(ki == 0 and kc == 0),
                            stop=(ki == 1 and kc == 3),
                            tile_position=(0, 0),
                        )
            C = work.tile([128, 2048], bf16, tag="C")
            for cc in range(4):
                nc.scalar.activation(
                    C[:, cc * 512:(cc + 1) * 512],
                    PT[:, cc * 512:(cc + 1) * 512],
                    AF.Tanh, bias=cbias_s[:, cc:cc + 1])
            lg = auxp.tile([64, 512], f32, tag="aux", name="lg")
            for cc in range(4):
                nc.tensor.matmul(
                    lg,
                    lhsT=fct_s[:, cc * 64:(cc + 1) * 64],
                    rhs=C[:, cc * 512:(cc + 1) * 512],
                    start=(cc == 0), stop=(cc == 3),
                    tile_position=(0, 0))
            lgs = work.tile([64, 512], f32, tag="lgs")
            nc.scalar.activation(lgs, lg, AF.Identity, bias=fbias_s[:, 0:1])
            nc.sync.dma_start(out=logT[:, 512 * i: 512 * (i + 1)], in_=lgs)

        for _ in range(repeat):
            emit_recurrence()
            emit_tail()

    nc.finalize()
    return nc


_prog_cache = {}


def _get_program(key):
    if key not in _prog_cache:
        _prog_cache[key] = build_program(*key)
    return _prog_cache[key]


def _gate_perm_half(g):
    """Rows of W (gate dim 2048, blocks [i,f,g,o] of 512) for half g in
    cell order [i f o g] x hid [g*256,(g+1)*256)."""
    perm = []
    for blk in (0, 1, 3, 2):   # i, f, o, g
        base = 512 * blk + HH * g
        perm.extend(range(base, base + HH))
    return np.array(perm)


def _prep_core_inputs(x, emb_table, inputs, bh, d, g, t_steps=T):
    perm = _gate_perm_half(g)
    Tn = t_steps
    xq = np.asarray(x[bh * BC:(bh + 1) * BC, :Tn]).astype(np.int64)
    if d == 1:
        xq = xq[:, ::-1]
    xs = xq.T.reshape(-1)                       # [Tn*BC] scan-order tokens
    ohv = np.zeros((64, (Tn + 1) * BC), dtype=np.float32)
    ohv[xs, np.arange(Tn * BC)] = 1.0           # last step stays zero

    W0 = np.asarray(inputs["W_f0"] if d == 0 else inputs["W_b0"])
    b0 = np.asarray(inputs["b_f0"] if d == 0 else inputs["b_b0"])
    W1 = np.asarray(inputs["W_f1"] if d == 0 else inputs["W_b1"])
    b1 = np.asarray(inputs["b_f1"] if d == 0 else inputs["b_b1"])
    W0h = W0[perm].astype(np.float32)           # [1024, 768]
    W1h = W1[perm].astype(np.float32)           # [1024, 1024]
    g0v = np.asarray(emb_table, np.float32) @ W0h[:, :EMB].T + b0[perm][None, :]
    wh0v = W0h[:, EMB:].T.reshape(4, 128, 1024)
    wx1v = W1h[:, :HS].T.reshape(4, 128, 1024)
    wh1v = W1h[:, HS:].T.reshape(4, 128, 1024)

    Wc = np.asarray(inputs["compress_W"], np.float32)     # [512, 1024]
    wc_own = Wc[:, d * HS:(d + 1) * HS]                   # own direction
    wc_prt = Wc[:, (1 - d) * HS:(2 - d) * HS]
    wcov = wc_own.T.reshape(4, 128, 512)                  # [hid-chunk, comp]
    wcpv = wc_prt.T.reshape(4, 128, 512)
    # fcT chunks: fc_W.T [512, 64] -> 4 chunks [128, 64] stacked in cols
    fctv = np.ascontiguousarray(
        np.asarray(inputs["fc_W"], np.float32).T.reshape(4, 128, 64)
        .transpose(1, 0, 2).reshape(128, 256))
    cbv = np.asarray(inputs["compress_b"], np.float32).reshape(4, 128).T
    fbv = np.asarray(inputs["fc_b"], np.float32).reshape(64, 1)

    inmap = {
        "ohT": ohv.astype(BF16),
        "g0tab": g0v.astype(BF16),
        "wh0": wh0v.astype(BF16),
        "wx1": wx1v.astype(BF16),
        "wh1": wh1v.astype(BF16),
        "wco": wcov.astype(BF16),
        "wcp": wcpv.astype(BF16),
        "fct": fctv.astype(BF16),
        "cbias": np.ascontiguousarray(cbv),
        "fbias": fbv,
    }
    if np.any(b1):
        inmap["b1row"] = b1[perm].reshape(1, 1024).astype(BF16)
    return inmap


def _run(inputs, trace=False, t_steps=T):
    from concourse.bass_utils import run_bass_kernel_spmd

    x = np.asarray(inputs["x"])
    emb_table = np.asarray(inputs["emb_table"], dtype=np.float32)
    with_b1 = bool(np.any(np.asarray(inputs["b_f1"]))
                   or np.any(np.asarray(inputs["b_b1"])))
    rep = int(os.environ.get("BLSTM_REPEAT", "1"))
    nc = _get_program((t_steps, rep, with_b1))

    in_maps = []
    for core in range(NCORES):
        bh, d, g = core // 4, (core % 4) // 2, core % 2
        im = _prep_core_inputs(x, emb_table, inputs, bh, d, g, t_steps)
        if with_b1 and "b1row" not in im:
            im["b1row"] = np.zeros((1, 1024), dtype=BF16)
        in_maps.append(im)

    res = run_bass_kernel_spmd(nc, in_maps, core_ids=list(range(NCORES)),
                               trace=trace)

    Tn = t_steps
    TS = Tn // 2
    out = np.empty((B, Tn, VS), dtype=np.float32)
    for bh in (0, 1):
        logF = res.results[bh * 4 + 0]["logT"]   # fwd g0 core
        logB = res.results[bh * 4 + 2]["logT"]   # bwd g0 core
        bsl = slice(bh * BC, (bh + 1) * BC)
        # fwd core: scan u = orig t in [0, TS)
        out[bsl, :TS] = logF.reshape(VS, TS, BC).transpose(2, 1, 0)
        # bwd core: scan u = orig Tn-1-u; its rows cover orig [TS, Tn)
        ob = logB.reshape(VS, TS, BC).transpose(2, 1, 0)   # [BC, u, VS]
        out[bsl, TS:] = ob[:, ::-1]
    return out, res


def kernel(**inputs):
    out, _ = _run(inputs, trace=False)
    return out


def kernel_profiled(**inputs):
    out, res = _run(inputs, trace=True)
    return out, res


# revision 19
# speedup vs baseline: 6.2674x; 1.1082x over previous
"""Bidirectional 2-layer LSTM (B=256, T=128, EMB=256, HS=512, VS=64) on 8 trn2 cores.

Sharding: core = (batch-half bh, direction d, gate-half g).
Each core runs the full 2-layer recurrence for its 128 batch rows and
direction, computing HALF the gate/hidden dims (hid [g*256,(g+1)*256) of
both layers); gate-half pairs AllGather their h-halves every step.

Per-iteration u (uniform, u = 0..T):
  - 26 matmuls (K<=128, M=128, N=512) accumulate L0 gates (step u) and
    L1 gates (step u-1) into one PSUM tile [128, 2048] =
    [i0 f0 o0 g0 | i1 f1 o1 g1] (own hid-half, 256 each).
  - 6 fused elementwise ops compute BOTH layers' LSTM cells via strided
    3-D APs (sigmoid, tanh(g), i*g|f*c, add, tanh(c), o*tc).
  - H [128, 512] = [H0own|H1own] -> dram -> pair AllGather -> load full
    H [128,1024] -> ONE dma_start_transpose -> hT blocks for next step.
Tail: dir-pairs AllReduce(add) the late h-slot window; partner stream =
sum - own (keeps SPMD code uniform under time reversal); each core
computes compress+fc for its own-scan steps [0, T/2) (fwd cores cover
original t in [0,T/2), bwd cores cover [T/2, T)).
"""

import os
import sys
from contextlib import ExitStack

import numpy as np
import ml_dtypes

for _p in ("/opt/trn_rl_repo",):
    if _p not in sys.path and os.path.isdir(_p):
        sys.path.insert(0, _p)

os.environ.setdefault("JAX_COMPILATION_CACHE_DIR", "/tmp/jaxcache")
os.environ.setdefault("JAX_PERSISTENT_CACHE_MIN_COMPILE_TIME_SECS", "1")

B, T, VS, EMB, HS = 256, 128, 64, 256, 512
NCORES = 8
BC = 128           # batch rows per core
HH = 256           # hid per gate-half

BF16 = ml_dtypes.bfloat16

# core = bh*4 + d*2 + g
GATE_PAIRS = [[0, 1], [2, 3], [4, 5], [6, 7]]   # (bh,d,g0) <-> (bh,d,g1)
DIR_PAIRS = [[0, 2], [1, 3], [4, 6], [5, 7]]    # (bh,f,g) <-> (bh,b,g)

# dmat block j of transposed full-H [128, 8, 128]:
# HF = [H0(g0) H1(g0) H0(g1) H1(g1)] in 256-col slabs ->
# blocks: [h0c0 h0c1 h1c0 h1c1 h0c2 h0c3 h1c2 h1c3]
BH0 = [0, 1, 4, 5]
BH1 = [2, 3, 6, 7]


def build_program(t_steps=T, repeat=1, with_b1=False, fp8=False, tail8=False):
    import concourse.bass as bass  # noqa: F401
    import concourse.mybir as mybir
    import concourse.tile as tile
    from concourse import bacc

    f32 = mybir.dt.float32
    bf16 = mybir.dt.bfloat16
    fp8e4 = mybir.dt.float8e4
    DR = mybir.MatmulPerfMode.DoubleRow
    AF = mybir.ActivationFunctionType
    Tn = t_steps
    TS = Tn // 2                  # own-scan tail steps per core
    NG = TS // 4                  # tail groups (4 steps = 512 rows each)

    nc = bacc.Bacc()

    # ---- I/O ----
    ohT = nc.dram_tensor("ohT", [64, (Tn + 1) * BC], bf16, kind="ExternalInput")
    g0tab = nc.dram_tensor("g0tab", [64, 1024], bf16, kind="ExternalInput")
    if fp8:
        # DoubleRow pair tiles: [A n0 | B n0 | A n1 | B n1] per chunk pair
        wh0 = nc.dram_tensor("wh0", [2, 128, 2048], fp8e4, kind="ExternalInput")
        wx1 = nc.dram_tensor("wx1", [2, 128, 2048], fp8e4, kind="ExternalInput")
        wh1 = nc.dram_tensor("wh1", [2, 128, 2048], fp8e4, kind="ExternalInput")
    else:
        wh0 = nc.dram_tensor("wh0", [4, 128, 1024], bf16, kind="ExternalInput")
        wx1 = nc.dram_tensor("wx1", [4, 128, 1024], bf16, kind="ExternalInput")
        wh1 = nc.dram_tensor("wh1", [4, 128, 1024], bf16, kind="ExternalInput")
    if tail8:
        # tail compress pairs: [A allcc | B allcc]
        wco = nc.dram_tensor("wco", [2, 128, 1024], fp8e4, kind="ExternalInput")
        wcp = nc.dram_tensor("wcp", [2, 128, 1024], fp8e4, kind="ExternalInput")
    else:
        wco = nc.dram_tensor("wco", [4, 128, 512], bf16, kind="ExternalInput")
        wcp = nc.dram_tensor("wcp", [4, 128, 512], bf16, kind="ExternalInput")
    fct = nc.dram_tensor("fct", [128, 256], bf16, kind="ExternalInput")
    cbias = nc.dram_tensor("cbias", [128, 4], f32, kind="ExternalInput")
    fbias = nc.dram_tensor("fbias", [64, 1], f32, kind="ExternalInput")
    if with_b1:
        b1row = nc.dram_tensor("b1row", [1, 1024], bf16, kind="ExternalInput")
    logT = nc.dram_tensor("logT", [64, TS * BC], f32, kind="ExternalOutput")

    # internal dram
    hin = nc.dram_tensor("hin", [Tn + 1, 128, 512], bf16)
    hout = nc.dram_tensor("hout", [Tn + 1, 2, 128, 512], bf16)
    arout = nc.dram_tensor("arout", [TS, 2, 128, 512], bf16)

    if os.environ.get("BLSTM_NULL", "0") == "1":
        with tile.TileContext(nc) as tc, ExitStack() as ctx:
            pool = ctx.enter_context(tc.tile_pool(name="np", bufs=1))
            z = pool.tile([64, 512], f32, name="z")
            nc.vector.memset(z, 0.0)
            nc.sync.dma_start(out=logT[:, 0:512], in_=z)
        nc.finalize()
        return nc

    with tile.TileContext(nc) as tc, ExitStack() as ctx:
        wpool = ctx.enter_context(tc.tile_pool(name="weights", bufs=1))
        spool = ctx.enter_context(tc.tile_pool(name="state", bufs=1))
        work = ctx.enter_context(tc.tile_pool(name="work", bufs=1))
        gpool = ctx.enter_context(tc.tile_pool(name="gp", bufs=1, space="PSUM"))
        auxp = ctx.enter_context(tc.tile_pool(name="auxp", bufs=1, space="PSUM"))

        # ---- load weights ----
        def load(dram, n, cols, tag):
            tiles = []
            for k in range(n):
                t_ = wpool.tile([128, cols], bf16, tag=f"{tag}{k}", name=f"{tag}{k}")
                nc.sync.dma_start(out=t_, in_=dram[k])
                tiles.append(t_)
            return tiles

        ohT_s = wpool.tile([64, (Tn + 1) * BC], bf16, tag="ohT")
        nc.sync.dma_start(out=ohT_s, in_=ohT[:, :])
        g0tab_s = wpool.tile([64, 1024], bf16, tag="g0tab")
        nc.sync.dma_start(out=g0tab_s, in_=g0tab[:, :])

        def load8(dram, n, cols, tag):
            tiles = []
            for k in range(n):
                t_ = wpool.tile([128, cols], fp8e4, tag=f"{tag}{k}", name=f"{tag}{k}")
                nc.sync.dma_start(out=t_, in_=dram[k])
                tiles.append(t_)
            return tiles

        if fp8:
            wh0_s = load8(wh0, 2, 2048, "wh0")
            wx1_s = load8(wx1, 2, 2048, "wx1")
            wh1_s = load8(wh1, 2, 2048, "wh1")
        else:
            wh0_s = load(wh0, 4, 1024, "wh0")
            wx1_s = load(wx1, 4, 1024, "wx1")
            wh1_s = load(wh1, 4, 1024, "wh1")
        if tail8:
            wco_s = load8(wco, 2, 1024, "wco")
            wcp_s = load8(wcp, 2, 1024, "wcp")
        else:
            wco_s = load(wco, 4, 512, "wco")
            wcp_s = load(wcp, 4, 512, "wcp")
        fct_s = wpool.tile([128, 256], bf16, tag="fct")
        nc.sync.dma_start(out=fct_s, in_=fct[:, :])
        cbias_s = wpool.tile([128, 4], f32, tag="cbias")
        nc.sync.dma_start(out=cbias_s, in_=cbias[:, :])
        fbias_s = wpool.tile([64, 1], f32, tag="fbias")
        nc.sync.dma_start(out=fbias_s, in_=fbias[:, :])
        if with_b1:
            b1_s = wpool.tile([1, 1024], bf16, tag="b1row")
            nc.sync.dma_start(out=b1_s, in_=b1row[:, :])
            ones_s = wpool.tile([1, 128], bf16, tag="ones")
            nc.vector.memset(ones_s, 1.0)

        # ---- state ----
        # X: [G0(256) c0(256) G1(256) c1(256)] fp32
        X = spool.tile([128, 1024], f32, tag="X")
        hT_ring = [spool.tile([128, 1024], bf16, tag=f"hT{i}", name=f"hT{i}")
                   for i in range(3)]
        if fp8:
            hT8_ring = [spool.tile([128, 1024], fp8e4, tag=f"hT8{i}",
                                   name=f"hT8{i}") for i in range(3)]

        def init_state():
            nc.vector.memset(X, 0.0)
            nc.vector.memset(hT_ring[2], 0.0)
            if fp8:
                nc.vector.memset(hT8_ring[2], 0.0)

        def gates_mms(gp, u, hT):
            """Gate matmuls: L0 gates (step u) into cols 0:1024,
            L1 gates (step u-1) into cols 1024:2048."""
            xstat = (ohT_s[:, u * BC:(u + 1) * BC], g0tab_s, None)
            if fp8:
                hv = hT.rearrange("p (j b) -> p j b", j=8)
                # DoubleRow pairs: h0 = blocks (0,1),(4,5); h1 = (2,3),(6,7)
                reg0 = [xstat]
                reg0 += [(hv[:, 4 * P:4 * P + 2, :], wh0_s[P], DR)
                         for P in (0, 1)]
                reg1 = [(hv[:, 4 * P:4 * P + 2, :], wx1_s[P], DR)
                        for P in (0, 1)]
                reg1 += [(hv[:, 4 * P + 2:4 * P + 4, :], wh1_s[P], DR)
                         for P in (0, 1)]
            else:
                reg0 = [xstat]
                reg0 += [(hT[:, BH0[kc] * 128:BH0[kc] * 128 + 128],
                          wh0_s[kc], None) for kc in range(4)]
                reg1 = [(hT[:, BH0[kc] * 128:BH0[kc] * 128 + 128],
                         wx1_s[kc], None) for kc in range(4)]
                reg1 += [(hT[:, BH1[kc] * 128:BH1[kc] * 128 + 128],
                          wh1_s[kc], None) for kc in range(4)]
            if with_b1:
                reg1.append((ones_s, b1_s, None))
            for base, stats in ((0, reg0), (1024, reg1)):
                nk = len(stats)
                for kid, (lhs, w, pm) in enumerate(stats):
                    for n in (0, 1):
                        if pm is DR:
                            rhs = w.rearrange("p (t i n) -> p t i n",
                                              t=2, i=2)[:, n]
                        else:
                            rhs = w[:, 512 * n: 512 * n + 512]
                        nc.tensor.matmul(
                            gp[:, base + 512 * n: base + 512 * n + 512],
                            lhsT=lhs,
                            rhs=rhs,
                            start=(kid == 0),
                            stop=(kid == nk - 1),
                            perf_mode=pm,
                            tile_position=(0, 0),
                        )

        def cell(gp):
            """Fused 2-layer cell; returns H [128, 512] = [H0own|H1own]."""
            gv = gp.rearrange("p (j c) -> p j c", j=2)       # [128, 2, 1024]
            S = work.tile([128, 1536], bf16, tag="S")
            Sv = S.rearrange("p (j c) -> p j c", j=2)        # [128, 2, 768]
            nc.scalar.activation(Sv, gv[:, :, 0:768], AF.Sigmoid)
            Xv = X.rearrange("p (j c) -> p j c", j=2)        # [128, 2, 512]
            nc.scalar.activation(Xv[:, :, 0:256], gv[:, :, 768:1024], AF.Tanh)
            P = work.tile([128, 1024], f32, tag="P")
            Pv = P.rearrange("p (j c) -> p j c", j=2)        # [128, 2, 512]
            nc.vector.tensor_mul(Pv, Sv[:, :, 0:512], Xv)
            nc.vector.tensor_add(Xv[:, :, 256:512], Pv[:, :, 0:256],
                                 Pv[:, :, 256:512])
            TC = work.tile([128, 512], bf16, tag="TC")
            TCv = TC.rearrange("p (j c) -> p j c", j=2)      # [128, 2, 256]
            nc.scalar.activation(TCv, Xv[:, :, 256:512], AF.Tanh)
            H = work.tile([128, 512], bf16, tag="H")
            Hv = H.rearrange("p (j c) -> p j c", j=2)
            nc.vector.tensor_mul(Hv, Sv[:, :, 512:768], TCv)
            return H

        def emit_recurrence():
            init_state()
            for u in range(Tn + 1):
                gp = gpool.tile([128, 2048], f32, tag="gp", name="gp")
                ring = hT8_ring if fp8 else hT_ring
                gates_mms(gp, u, ring[(u - 1) % 3] if u > 0 else ring[2])
                H = cell(gp)
                nc.sync.dma_start(out=hin[u], in_=H)
                nc.gpsimd.collective_compute(
                    "AllGather", mybir.AluOpType.bypass,
                    replica_groups=GATE_PAIRS,
                    ins=[hin[u]], outs=[hout[u]])
                HF = work.tile([128, 1024], bf16, tag="HF")
                nc.sync.dma_start(out=HF, in_=hout[u].rearrange("j p c -> p j c"))
                nc.sync.dma_start_transpose(
                    out=hT_ring[u % 3].rearrange("p (j b) -> p j b", j=8),
                    in_=HF)
                if fp8:
                    nc.vector.tensor_copy(hT8_ring[u % 3], hT_ring[u % 3])

        def emit_tail():
            # AllReduce the late window: slots [TS+1, Tn+1) (TS slots).
            nc.gpsimd.collective_compute(
                "AllReduce", mybir.AluOpType.add,
                replica_groups=DIR_PAIRS,
                ins=[hin_window()], outs=[arout[:, :, :, :]])
            for i in range(NG):
                emit_tail_group(i)

        def hin_window():
            # copy hout slots [TS+1 : Tn+1) to arin via dram->dram dma?
            # cheaper: AllReduce directly on a hout slice.
            return hout[TS + 1: Tn + 1]

        def emit_tail_group(i):
            # own-scan steps s = 4i .. 4i+3; 512 rows.
            # Load FULL h slots [2,128,512] -> [128, 1024] per step
            # (h1 blocks selected post-transpose via BH1 strides).
            HT1 = work.tile([128, 4096], bf16, tag="HT1")   # own-dir rows
            ART = work.tile([128, 4096], bf16, tag="ART")   # fwd+bwd sum
            OWT = work.tile([128, 4096], bf16, tag="OWT")   # own window
            for k in range(4):
                s = 4 * i + k
                cs = slice(k * 1024, (k + 1) * 1024)
                rr = lambda t_: t_.rearrange("j p c -> p j c")
                nc.sync.dma_start(out=HT1[:, cs], in_=rr(hout[s + 1]))
                nc.sync.dma_start(out=ART[:, cs], in_=rr(arout[TS - 1 - s]))
                nc.sync.dma_start(out=OWT[:, cs], in_=rr(hout[Tn - s]))
            PRT = work.tile([128, 4096], bf16, tag="PRT")   # partner rows
            nc.vector.tensor_sub(PRT, ART, OWT)
            hTo = work.tile([128, 4096], bf16, tag="hTo")
            nc.sync.dma_start_transpose(
                out=hTo.rearrange("p (j b) -> p j b", j=32), in_=HT1)
            hTp = work.tile([128, 4096], bf16, tag="hTp")
            nc.sync.dma_start_transpose(
                out=hTp.rearrange("p (j b) -> p j b", j=32), in_=PRT)
            # compress matmuls: PT[cc] [128, 512] for 4 comp chunks
            PT = auxp.tile([128, 2048], f32, tag="aux", name="PT")
            if tail8:
                # repack h1 blocks into contiguous DR layout [P, i, s, b]
                def cast8(srcT, tag):
                    t8 = work.tile([128, 2048], fp8e4, tag=tag)
                    sv = srcT.rearrange("p (s j b) -> p j s b", s=4, j=8)
                    ov = t8.rearrange("p (P i s b) -> p P i s b", P=2, i=2, s=4)
                    for P in (0, 1):
                        nc.vector.tensor_copy(ov[:, P], sv[:, 2 + 4 * P:4 + 4 * P])
                    return t8
                hTo8 = cast8(hTo, "hTo8")
                hTp8 = cast8(hTp, "hTp8")
                for cc in range(4):
                    for ki, (wt, hv) in enumerate(
                            [(wco_s, hTo8), (wcp_s, hTp8)]):
                        for P in (0, 1):
                            lhsT = wt[P][:, cc * 256:(cc + 1) * 256].rearrange(
                                "p (i m) -> p i m", i=2)
                            rhs = hv[:, P * 1024:(P + 1) * 1024].rearrange(
                                "p (i n) -> p i n", i=2)
                            nc.tensor.matmul(
                                PT[:, cc * 512:(cc + 1) * 512],
                                lhsT=lhsT,
                                rhs=rhs,
                                start=(ki == 0 and P == 0),
                                stop=(ki == 1 and P == 1),
                                perf_mode=DR,
                                tile_position=(0, 0),
                            )
            else:
                hToV = hTo.rearrange("p (s j b) -> p s j b", s=4, j=8)
                hTpV = hTp.rearrange("p (s j b) -> p s j b", s=4, j=8)
                for cc in range(4):
                    for ki, (wt, hv) in enumerate(
                            [(wco_s, hToV), (wcp_s, hTpV)]):
                        for kc in range(4):
                            nc.tensor.matmul(
                                PT[:, cc * 512:(cc + 1) * 512],
                                lhsT=wt[kc][:, cc * 128:(cc + 1) * 128],
                                rhs=hv[:, :, BH1[kc], :],
                                start=(ki == 0 and kc == 0),
                                stop=(ki == 1 and kc == 3),
                                tile_position=(0, 0),
                            )
            C = work.tile([128, 2048], bf16, tag="C")
            for cc in range(4):
                nc.scalar.activation(
                    C[:, cc * 512:(cc + 1) * 512],
                    PT[:, cc * 512:(cc + 1) * 512],
                    AF.Tanh, bias=cbias_s[:, cc:cc + 1])
            lg = auxp.tile([64, 512], f32, tag="aux", name="lg")
            for cc in range(4):
                nc.tensor.matmul(
                    lg,
                    lhsT=fct_s[:, cc * 64:(cc + 1) * 64],
                    rhs=C[:, cc * 512:(cc + 1) * 512],
                    start=(cc == 0), stop=(cc == 3),
                    tile_position=(0, 0))
            lgs = work.tile([64, 512], f32, tag="lgs")
            nc.scalar.activation(lgs, lg, AF.Identity, bias=fbias_s[:, 0:1])
            nc.sync.dma_start(out=logT[:, 512 * i: 512 * (i + 1)], in_=lgs)

        for _ in range(repeat):
            emit_recurrence()
            emit_tail()

    nc.finalize()
    return nc


_prog_cache = {}


def _get_program(key):
    if key not in _prog_cache:
        _prog_cache[key] = build_program(*key)
    return _prog_cache[key]


def _drpackc(w4):
    # [4, 128, 512] -> pairs [2, 128, 1024]: per cc: [A_cc(128) | B_cc(128)]
    FP8 = ml_dtypes.float8_e4m3
    out = np.empty((2, 128, 1024), np.float32)
    for P in (0, 1):
        A, Bc = w4[2 * P], w4[2 * P + 1]
        cols = []
        for cc in range(4):
            cols += [A[:, cc * 128:(cc + 1) * 128], Bc[:, cc * 128:(cc + 1) * 128]]
        out[P] = np.concatenate(cols, axis=1)
    return out.astype(FP8)


def _gate_perm_half(g):
    """Rows of W (gate dim 2048, blocks [i,f,g,o] of 512) for half g in
    cell order [i f o g] x hid [g*256,(g+1)*256)."""
    perm = []
    for blk in (0, 1, 3, 2):   # i, f, o, g
        base = 512 * blk + HH * g
        perm.extend(range(base, base + HH))
    return np.array(perm)


def _prep_core_inputs(x, emb_table, inputs, bh, d, g, t_steps=T, fp8=False, tail8=False):
    perm = _gate_perm_half(g)
    Tn = t_steps
    xq = np.asarray(x[bh * BC:(bh + 1) * BC, :Tn]).astype(np.int64)
    if d == 1:
        xq = xq[:, ::-1]
    xs = xq.T.reshape(-1)                       # [Tn*BC] scan-order tokens
    ohv = np.zeros((64, (Tn + 1) * BC), dtype=np.float32)
    ohv[xs, np.arange(Tn * BC)] = 1.0           # last step stays zero

    W0 = np.asarray(inputs["W_f0"] if d == 0 else inputs["W_b0"])
    b0 = np.asarray(inputs["b_f0"] if d == 0 else inputs["b_b0"])
    W1 = np.asarray(inputs["W_f1"] if d == 0 else inputs["W_b1"])
    b1 = np.asarray(inputs["b_f1"] if d == 0 else inputs["b_b1"])
    W0h = W0[perm].astype(np.float32)           # [1024, 768]
    W1h = W1[perm].astype(np.float32)           # [1024, 1024]
    g0v = np.asarray(emb_table, np.float32) @ W0h[:, :EMB].T + b0[perm][None, :]
    wh0v = W0h[:, EMB:].T.reshape(4, 128, 1024)
    wx1v = W1h[:, :HS].T.reshape(4, 128, 1024)
    wh1v = W1h[:, HS:].T.reshape(4, 128, 1024)

    Wc = np.asarray(inputs["compress_W"], np.float32)     # [512, 1024]
    wc_own = Wc[:, d * HS:(d + 1) * HS]                   # own direction
    wc_prt = Wc[:, (1 - d) * HS:(2 - d) * HS]
    wcov = wc_own.T.reshape(4, 128, 512)                  # [hid-chunk, comp]
    wcpv = wc_prt.T.reshape(4, 128, 512)

    if fp8:
        FP8 = ml_dtypes.float8_e4m3

        def drpack(w4):
            # [4, 128, 1024] -> pairs [2, 128, 2048]: [A n0 | B n0 | A n1 | B n1]
            out = np.empty((2, 128, 2048), np.float32)
            for P in (0, 1):
                A, Bc = w4[2 * P], w4[2 * P + 1]
                out[P] = np.concatenate(
                    [A[:, :512], Bc[:, :512], A[:, 512:], Bc[:, 512:]], axis=1)
            return out.astype(FP8)

        wh0v, wx1v, wh1v = drpack(wh0v), drpack(wx1v), drpack(wh1v)
    if tail8:
        wcov, wcpv = _drpackc(wcov), _drpackc(wcpv)
    # fcT chunks: fc_W.T [512, 64] -> 4 chunks [128, 64] stacked in cols
    fctv = np.ascontiguousarray(
        np.asarray(inputs["fc_W"], np.float32).T.reshape(4, 128, 64)
        .transpose(1, 0, 2).reshape(128, 256))
    cbv = np.asarray(inputs["compress_b"], np.float32).reshape(4, 128).T
    fbv = np.asarray(inputs["fc_b"], np.float32).reshape(64, 1)

    wdt = (lambda a: a) if fp8 else (lambda a: a.astype(BF16))
    cdt = (lambda a: a) if tail8 else (lambda a: a.astype(BF16))
    inmap = {
        "ohT": ohv.astype(BF16),
        "g0tab": g0v.astype(BF16),
        "wh0": wdt(wh0v),
        "wx1": wdt(wx1v),
        "wh1": wdt(wh1v),
        "wco": cdt(wcov),
        "wcp": cdt(wcpv),
        "fct": fctv.astype(BF16),
        "cbias": np.ascontiguousarray(cbv),
        "fbias": fbv,
    }
    if np.any(b1):
        inmap["b1row"] = b1[perm].reshape(1, 1024).astype(BF16)
    return inmap


def _run(inputs, trace=False, t_steps=T):
    from concourse.bass_utils import run_bass_kernel_spmd

    x = np.asarray(inputs["x"])
    emb_table = np.asarray(inputs["emb_table"], dtype=np.float32)
    with_b1 = bool(np.any(np.asarray(inputs["b_f1"]))
                   or np.any(np.asarray(inputs["b_b1"])))
    rep = int(os.environ.get("BLSTM_REPEAT", "1"))
    fp8 = os.environ.get("BLSTM_FP8", "0") == "1"
    tail8 = os.environ.get("BLSTM_FP8_TAIL", "0") == "1"
    nc = _get_program((t_steps, rep, with_b1, fp8, tail8))

    in_maps = []
    for core in range(NCORES):
        bh, d, g = core // 4, (core % 4) // 2, core % 2
        im = _prep_core_inputs(x, emb_table, inputs, bh, d, g, t_steps, fp8, tail8)
        if with_b1 and "b1row" not in im:
            im["b1row"] = np.zeros((1, 1024), dtype=BF16)
        in_maps.append(im)

    res = run_bass_kernel_spmd(nc, in_maps, core_ids=list(range(NCORES)),
                               trace=trace)

    Tn = t_steps
    TS = Tn // 2
    out = np.empty((B, Tn, VS), dtype=np.float32)
    for bh in (0, 1):
        logF = res.results[bh * 4 + 0]["logT"]   # fwd g0 core
        logB = res.results[bh * 4 + 2]["logT"]   # bwd g0 core
        bsl = slice(bh * BC, (bh + 1) * BC)
        # fwd core: scan u = orig t in [0, TS)
        out[bsl, :TS] = logF.reshape(VS, TS, BC).transpose(2, 1, 0)
        # bwd core: scan u = orig Tn-1-u; its rows cover orig [TS, Tn)
        ob = logB.reshape(VS, TS, BC).transpose(2, 1, 0)   # [BC, u, VS]
        out[bsl, TS:] = ob[:, ::-1]
    return out, res


def kernel(**inputs):
    out, _ = _run(inputs, trace=False)
    return out


def kernel_profiled(**inputs):
    out, res = _run(inputs, trace=True)
    return out, res
